# revision 1
# baseline (speedup 1.0000x reference)
"""DownSample (depthwise FIR [1,3,3,1]^2/64 pad-2, then 3x3 stride-2 conv + bias)
as a Trainium2 Bass kernel, data-parallel over batch across 8 NeuronCores.

Block-pipelined redesign of the staged baseline:
  - per (batch, ic-chunk) block: ACT ingest cast -> DVE vertical cascade
    (c1, c2, V: three [1,1] stages = [1,3,3,1]) -> horizontal [1,2,1] as two
    plain [1,1] cascade adds (s, M) with the outer [1,1] folded into the conv
    weights (w4, 12 taps) -- no scaled-copy pass (the baseline burned an ACT
    pass on M = u + 2V; s/M cascade computes the identical M).
  - s is split between GPSIMD (rows < POOL_ROWS) and DVE (rest): GPSIMD is
    otherwise idle and runs tensor_add at ~2.2 ns/elem.
  - conv for block i-1 (48 N=512 matmuls) runs while block i's FIR computes;
    M is triple-buffered, PSUM groups accumulate across the two ic-chunk
    blocks of a batch (8 banks total).
  - pad rows/cols of the fixed ping-pong buffers are zeroed once in the
    preamble, not per block.
  - the For_i timing loop wraps `reps` unrolled bodies so the per-iteration
    all-engine barrier amortizes.
"""

import numpy as np
import ml_dtypes

import concourse.bass as bass
import concourse.mybir as mybir
import concourse.tile as tile
from concourse.bass_utils import run_bass_kernel_spmd
from concourse.vector_clock import ScopedClock, VectorClock

# problem geometry (hardcoded per contract)
B_FULL, C, H, W = 16, 256, 64, 64
OC, OH, OW = 256, 32, 32
N_CORES = 8
BPC = B_FULL // N_CORES      # batches per core
KH, KQ = 3, 4                # folded conv taps (rows x cols)
NCH = C // 128               # input-channel chunks
NOB = OC // 128              # output-channel blocks
WV = W + 4                   # padded V row width
WS = W + 3                   # s row width (67)
WM = W + 2                   # M row width (66)
NV = H + 1                   # V/s/M row count (65)
POOL_ROWS = 0                # s rows on GPSIMD (rest DVE)
# (GPSIMD offload measured slower on HW than the cost model predicts --
#  bf16 tensor_tensor on Q7 + SBUF-port contention with DVE; keep off.)

F32 = mybir.dt.float32
BF16 = mybir.dt.bfloat16
COPY = mybir.ActivationFunctionType.Copy
IDENT = mybir.ActivationFunctionType.Identity


class SplitDrainTileContext(tile.TileContext):
    """walrus codegen caps sync-wait commands per instruction; the kernel-tail
    drain waits once per proc lane, which overflows once several DMA queues
    are used. Emit one single-wait SP nop per pending proc first; SP executes
    in order, so the drain itself then needs no waits."""

    def _drain_and_barrier(self, tick_clock, wait_clock):
        g = tick_clock.global_clock
        n = len(g)
        for p in range(n):
            if g[p] > 0:
                vec = [g[q] if q == p else 0 for q in range(n)]
                nop = self.nc.sync.nop()
                wait_clock.add_sem_waits(nop.ins, ScopedClock({None: VectorClock(vec)}))
        self.nc.sync.drain()
        self.nc.all_engine_barrier()
        assert self.sems is not None
        popped = self.nc._tile_sem_poison_stack.pop()
        assert popped is self._sem_poison
        self.nc.clear_and_free_semaphores(list(self.sems.allocated().values()))
        self.nc.all_engine_barrier()


def _split_excess_waits(nc: bass.Bass, max_waits: int = 1) -> None:
    """The TRN2 ISA (and walrus codegen) allows at most 2 sync-wait commands
    per instruction (1 for matmul), but Tile's wait assignment can attach
    more. Spill excess waits onto same-engine nops inserted immediately
    before the instruction."""
    for fn in nc.m.functions:
        for bb in fn.blocks:
            out = []
            changed = False
            for ins in bb.instructions:
                si = ins.sync_info
                waits = list(si.on_wait) if si else []
                if len(waits) > max_waits:
                    changed = True
                    excess, keep = waits[:-max_waits], waits[-max_waits:]
                    for k in range(0, len(excess), max_waits):
                        nop = mybir.InstNoOp(
                            name=f"{ins.name}-wsplit{k}", ins=[], outs=[],
                            engine=ins.engine)
                        nop.sync_info = mybir.SyncInfo(
                            on_wait=excess[k:k + max_waits], on_update=[])
                        nc.register_instruction(nop, overwrite=True)
                        out.append(nop)
                    ins.sync_info = mybir.SyncInfo(
                        on_wait=keep, on_update=list(si.on_update))
                out.append(ins)
            if changed:
                bb.instructions = out


def build_program(reps: int = 1, loop_n: int | None = None,
                  staggered: bool = False) -> bass.Bass:
    """One-core program; run SPMD on 8 cores. `reps` unrolls the body; with
    `loop_n` a single device-side For_i wraps all `reps` bodies."""
    nc = bass.Bass()
    xin = nc.declare_dram_parameter("x", [BPC, C, H, W], F32, isOutput=False)
    win = nc.declare_dram_parameter("w", [128, KH * KQ * NCH * NOB * 128], BF16,
                                    isOutput=False)
    bin_ = nc.declare_dram_parameter("b", [NOB, 128], F32, isOutput=False)
    yout = nc.declare_dram_parameter("y", [BPC, OC, OH, OW], F32, isOutput=True)

    blocks = [(b, c) for b in range(BPC) for c in range(NCH)]
    NB = len(blocks)

    with SplitDrainTileContext(nc) as tc:
        rotated = loop_n is not None
        with tc.tile_pool(name="const", bufs=1) as cpool, \
             tc.tile_pool(name="xf", bufs=3) as xfpool, \
             tc.tile_pool(name="fir", bufs=2) as fpool, \
             tc.tile_pool(name="vsm", bufs=3) as vpool, \
             tc.tile_pool(name="mp", bufs=4) as mpool, \
             tc.tile_pool(name="osb", bufs=3) as opool, \
             tc.tile_pool(name="ps", bufs=4 if rotated else 8,
                          space="PSUM") as pspool, \
             tc.tile_pool(name="psl", bufs=1, space="PSUM") as pslpool:

            w_sb = cpool.tile([128, KH * KQ * NCH * NOB * 128], BF16)
            nc.sync.dma_start(out=w_sb[:], in_=win[:])
            bias_sb = cpool.tile([128, NOB], F32)
            nc.sync.dma_start(out=bias_sb[:], in_=bin_[:].rearrange("o p -> p o"))

            # Column-polyphase layout: every image buffer stores even columns
            # in its left half and odd columns in its right half, so all DVE
            # adds stay packed (2x mode) AND the conv matmul rhs reads
            # CONTIGUOUS column windows (strided last-dim rhs measured ~25
            # ns/MM slower on HW).
            #   xb/c1/c2 [rows, 64]: cols 0:32 = x even cols, 32:64 = odd
            #   V [65, 68]: Ve = cols 0:34 (Vb even), Vo = 34:68 (Vb odd)
            #   s [65, 67]: Se = cols 0:34 (Sb even), So = 34:67 (Sb odd)
            #   m [65, 66]: Me = cols 0:33 (Mb even), Mo = 33:66 (Mb odd)
            # with Sb[g] = Vb[g]+Vb[g+1], Mb[g] = Sb[g]+Sb[g+1] as before:
            #   Se[j] = Ve[j]+Vo[j]      So[j] = Vo[j]+Ve[j+1]
            #   Me[j] = Se[j]+So[j]      Mo[j] = So[j]+Se[j+1]
            # conv tap q reads Mb[q+2ow]: q=0 -> Me[ow], q=1 -> Mo[ow],
            # q=2 -> Me[ow+1], q=3 -> Mo[ow+1].

            # fixed 3-deep rotations for the two pad-carrying buffers; pads
            # are zeroed once here instead of per block.
            HW2 = W // 2  # 32
            xb3f, v3f = [], []
            for j in range(3):
                xb = cpool.tile([128, (H + 4) * W], BF16, name=f"xb{j}")
                xb3 = xb[:].rearrange("p (h w) -> p h w", w=W)
                # full-tile zero: covers the pad rows AND gives iteration 0
                # of the rotated loop defined block-0 data
                nc.vector.memset(xb3[:, :, :], 0.0)
                xb3f.append(xb3)
                vb = cpool.tile([128, NV * WV], BF16, name=f"v{j}")
                v3 = vb[:].rearrange("p (h w) -> p h w", w=WV)
                # Vb zero pads: Ve[0]=Vb[0], Vo[0]=Vb[1], Ve[33]=Vb[66],
                # Vo[33]=Vb[67] -> polyphase cols 0, 33, 34, 67
                nc.vector.memset(v3[:, :, 0:1], 0.0)
                nc.vector.memset(v3[:, :, 33:35], 0.0)
                nc.vector.memset(v3[:, :, 67:68], 0.0)
                v3f.append(v3)

            xb3s: dict = {}
            v3s: dict = {}
            s3s: dict = {}
            m3s: dict = {}

            def emit_load(rep, i):
                b, ci = blocks[i]
                xb3 = xb3f[(rep * NB + i) % 3]
                for (r0, nr) in ((0, 32), (32, 32)):
                    xf = xfpool.tile([128, nr * W], F32, tag="xf",
                                     name=f"xf{rep}_{i}_{r0}")
                    nc.sync.dma_start(
                        out=xf[:],
                        in_=xin[b, ci * 128:(ci + 1) * 128, r0:r0 + nr, :]
                        .rearrange("c h w -> c (h w)"))
                    xf3 = xf[:].rearrange("p (h w) -> p h w", w=W)
                    # cast + polyphase split (ACT is stride-insensitive)
                    nc.scalar.activation(
                        xb3[:, 2 + r0:2 + r0 + nr, 0:HW2],
                        xf3[:, :, 0:W:2], COPY)
                    nc.scalar.activation(
                        xb3[:, 2 + r0:2 + r0 + nr, HW2:W],
                        xf3[:, :, 1:W:2], COPY)
                xb3s[i % 2] = xb3

            def emit_fir_v(rep, i):
                xb3 = xb3s[i % 2]
                c1 = fpool.tile([128, (H + 3) * W], BF16, tag="c1",
                                name=f"c1_{rep}_{i}")
                c13 = c1[:].rearrange("p (h w) -> p h w", w=W)
                c2 = fpool.tile([128, (H + 2) * W], BF16, tag="c2",
                                name=f"c2_{rep}_{i}")
                c23 = c2[:].rearrange("p (h w) -> p h w", w=W)
                v3 = v3f[(rep * NB + i) % 3]
                # vertical [1,3,3,1] as three [1,1] cascade stages, on the
                # even and odd column halves (same total elements)
                nc.vector.tensor_add(c13[:, 0:H + 3, :],
                                     xb3[:, 0:H + 3, :],
                                     xb3[:, 1:H + 4, :])
                nc.vector.tensor_add(c23[:, 0:H + 2, :],
                                     c13[:, 0:H + 2, :],
                                     c13[:, 1:H + 3, :])
                # V data: Ve cols 1..32 <- c2 even half, Vo cols 1..32 <- odd
                nc.vector.tensor_add(v3[:, 0:NV, 1:33],
                                     c23[:, 0:NV, 0:HW2],
                                     c23[:, 1:NV + 1, 0:HW2])
                nc.vector.tensor_add(v3[:, 0:NV, 35:67],
                                     c23[:, 0:NV, HW2:W],
                                     c23[:, 1:NV + 1, HW2:W])
                v3s[i % 2] = v3

            def emit_fir_s(rep, i):
                v3 = v3s[i % 2]
                sb = vpool.tile([128, NV * WS], BF16, tag="s",
                                name=f"s_{rep}_{i}")
                s3 = sb[:].rearrange("p (h w) -> p h w", w=WS)
                # Se[j] = Ve[j]+Vo[j] (j 0..33); So[j] = Vo[j]+Ve[j+1] (0..32)
                if POOL_ROWS > 0:
                    nc.gpsimd.tensor_add(s3[:, 0:POOL_ROWS, 0:34],
                                         v3[:, 0:POOL_ROWS, 0:34],
                                         v3[:, 0:POOL_ROWS, 34:68])
                if POOL_ROWS < NV:
                    nc.vector.tensor_add(s3[:, POOL_ROWS:NV, 0:34],
                                         v3[:, POOL_ROWS:NV, 0:34],
                                         v3[:, POOL_ROWS:NV, 34:68])
                nc.vector.tensor_add(s3[:, 0:NV, 34:67],
                                     v3[:, 0:NV, 34:67],
                                     v3[:, 0:NV, 1:34])
                s3s[i % 2] = s3

            def emit_fir_m(rep, i, mb=None):
                s3 = s3s[i % 2]
                if mb is None:
                    mb = mpool.tile([128, NV * WM], BF16, tag="m",
                                    name=f"m_{rep}_{i}")
                m3 = mb[:].rearrange("p (h w) -> p h w", w=WM)
                # Me[j] = Se[j]+So[j] (j 0..32); Mo[j] = So[j]+Se[j+1] (0..32)
                nc.vector.tensor_add(m3[:, 0:NV, 0:33],
                                     s3[:, 0:NV, 0:33],
                                     s3[:, 0:NV, 34:67])
                nc.vector.tensor_add(m3[:, 0:NV, 33:66],
                                     s3[:, 0:NV, 34:67],
                                     s3[:, 0:NV, 1:34])
                m3s[i % 3] = m3

            def emit_conv(pss, i, skip_check=False):
                b, ci = blocks[i]
                mm = m3s[i % 3]
                for grp in range(2):
                    for ocb in range(NOB):
                        for kh in range(KH):
                            # even-parity taps first: they read only Me,
                            # which the FIR writes before Mo
                            for q in (0, 2, 1, 3):
                                widx = ((kh * KQ + q) * NCH + ci) * NOB + ocb
                                lhsT = w_sb[:, widx * 128:(widx + 1) * 128]
                                first = ci == 0 and kh == 0 and q == 0
                                last = (ci == NCH - 1 and kh == KH - 1
                                        and q == KQ - 1)
                                c0 = 33 * (q % 2) + q // 2
                                rhs = mm[:, 32 * grp + kh:32 * grp + kh + 31:2,
                                         c0:c0 + 32]
                                out3 = pss[(b, ocb, grp)][:].rearrange(
                                    "p (h w) -> p h w", w=OW)
                                nc.tensor.matmul(out3, lhsT, rhs,
                                                 start=first, stop=last,
                                                 skip_group_check=skip_check)

            def emit_evac(rep, pss, b):
                for ocb in range(NOB):
                    osb = opool.tile([128, OH * OW], F32, tag="osb",
                                     name=f"osb{rep}_{b}_{ocb}")
                    for grp in range(2):
                        nc.scalar.activation(
                            osb[:, grp * 512:(grp + 1) * 512],
                            pss[(b, ocb, grp)][:],
                            IDENT, bias=bias_sb[:, ocb:ocb + 1])
                    nc.sync.dma_start(
                        out=yout[b, ocb * 128:(ocb + 1) * 128, :, :]
                        .rearrange("c h w -> c (h w)"),
                        in_=osb[:])

            def emit_warmup(n_mm, pss):
                # PE warmup garbage matmuls into a bank the real taps later
                # clear (start=True); keeps HAM warming during the FIR lead-in.
                wout = pss[(0, 0, 0)][:].rearrange("p (h w) -> p h w", w=OW)
                rhs = xb3s[0][:, 0:16, 0:32]
                for _ in range(n_mm):
                    nc.tensor.matmul(wout, w_sb[:, 0:128], rhs,
                                     start=True, stop=True)

            # In the loop build (`rotated`), the last rep's final conv +
            # b1 evacuation move to the TOP of the body: they consume the
            # loop-carried M tile / PSUM partials written at the body's end,
            # so right after the For_i all-engine barrier every engine has
            # immediate work (no serial fill behind the previous tail).
            m_last = None
            psl = None
            if rotated:
                m_last = cpool.tile([128, NV * WM], BF16, name="mlast")
                nc.vector.memset(m_last[:], 0.0)
                psl = {(1, ocb, grp): pslpool.tile(
                    [128, 512], F32, name=f"psl_{ocb}_{grp}")
                    for ocb in range(NOB) for grp in range(2)}

            _loop = None
            if loop_n is not None:
                _loop = tc.For_i(0, loop_n, 1, hint_engines=(
                    mybir.EngineType.PE, mybir.EngineType.DVE,
                    mybir.EngineType.Activation, mybir.EngineType.Pool),
                    staggered_reset=staggered)
                _loop.__enter__()

            if rotated:
                # finish the previous iteration's b1: stop-half of its conv
                # accumulation group (PSUM state persists across the
                # barrier); the evac is deferred past rep 0's first casts.
                # Iteration 0 computes garbage into y[b1], overwritten by
                # every later iteration.
                m3s[(NB - 1) % 3] = m_last[:].rearrange(
                    "p (h w) -> p h w", w=WM)
                emit_conv(psl, NB - 1, skip_check=True)

            # b1's evacuation of inner reps is deferred into the NEXT rep's
            # stream (after its first ingest cast) so the next rep's ACT/DVE
            # work is not queued behind an evac waiting on this rep's conv.
            # In rotated builds the block-0 loads are ALSO shifted one rep
            # back (the last rep prefetches the next ITERATION's block 0
            # across the barrier into the xb ring -- requires reps*NB % 3
            # == 0 so the ring phase matches).
            if rotated:
                assert (reps * NB) % 3 == 0, "rotated loads need reps%3==0"
            pending_evac = ("rot", psl) if rotated else None
            for rep in range(reps):
                last = rep == reps - 1
                pss = {(b, ocb, grp): (
                    psl[(b, ocb, grp)] if (rotated and b == 1)
                    else pspool.tile([128, 512], F32, tag="ps",
                                     name=f"ps{rep}_{b}_{ocb}_{grp}"))
                    for b in range(BPC) for ocb in range(NOB)
                    for grp in range(2)}
                if not rotated:
                    emit_load(rep, 0)
                elif rep == 0:
                    # block-0 data was prefetched by the previous iteration's
                    # tail; just point xb3s at the right ring slot.
                    xb3s[0] = xb3f[0]
                if rep == 0 and loop_n is None:
                    emit_warmup(12, pss)
                for i in range(NB):
                    if i >= 1:
                        # M(i-1) first: it only needs s(i-1), and emitting it
                        # ahead of block i's cascade lets PE start conv(i-1)
                        # a couple of microseconds earlier.
                        emit_fir_m(rep, i - 1)
                        emit_conv(pss, i - 1, skip_check=rotated and i - 1 >= 2)
                    if i + 1 < NB:
                        emit_load(rep, i + 1)
                    elif rotated:
                        if not last:
                            emit_load(rep + 1, 0)
                        else:
                            emit_load(0, 0)  # next iteration's block 0
                    if i == 0 and pending_evac is not None:
                        # deferred b1 evac AFTER this segment's casts: the
                        # in-order ACT queue must not hold the next block's
                        # ingest cast behind an evac that waits on a conv.
                        emit_evac(*pending_evac, 1)
                        pending_evac = None
                    if i == 2:
                        emit_evac(rep, pss, 0)
                    emit_fir_v(rep, i)
                    emit_fir_s(rep, i)
                if rotated and last:
                    # M(3) lands in the loop-carried tile; its conv + evac
                    # run at the next iteration's top.
                    emit_fir_m(rep, NB - 1, mb=m_last)
                else:
                    emit_fir_m(rep, NB - 1)
                    emit_conv(pss, NB - 1, skip_check=rotated)
                    pending_evac = (rep, pss)
            if pending_evac is not None:
                emit_evac(*pending_evac, 1)

            if _loop is not None:
                _loop.__exit__(None, None, None)
    _split_excess_waits(nc)
    return nc


def prep_weights(w: np.ndarray) -> np.ndarray:
    """w [256,256,3,3] f32 -> [128, 48*128] bf16 lhsT tiles.
    Folds horizontal [1,1] and the 1/64 FIR normalization:
    w4[q] coefficients multiply M[2ow+q-1]."""
    w = np.asarray(w, np.float32)
    w4 = np.zeros((OC, C, KH, KQ), np.float32)
    w4[:, :, :, 0] = w[:, :, :, 0]
    w4[:, :, :, 1] = w[:, :, :, 0] + w[:, :, :, 1]
    w4[:, :, :, 2] = w[:, :, :, 1] + w[:, :, :, 2]
    w4[:, :, :, 3] = w[:, :, :, 2]
    w4 *= 1.0 / 64.0
    # -> [kh, q, c_chunk, ocb, ic(128), oc(128)]
    t = w4.reshape(NOB, 128, NCH, 128, KH, KQ).transpose(4, 5, 2, 0, 3, 1)
    t = np.ascontiguousarray(t).reshape(KH * KQ * NCH * NOB, 128, 128)
    return t.transpose(1, 0, 2).reshape(128, -1).astype(ml_dtypes.bfloat16)


_NC_CACHE: dict = {}


def _get_nc(reps: int = 1, loop_n: int | None = None,
            staggered: bool = False) -> bass.Bass:
    key = (reps, loop_n, staggered)
    if key not in _NC_CACHE:
        _NC_CACHE[key] = build_program(reps, loop_n, staggered)
    return _NC_CACHE[key]


def make_in_maps(x: np.ndarray, w: np.ndarray, b: np.ndarray):
    wp = prep_weights(w)
    bp = np.asarray(b, np.float32).reshape(NOB, 128)
    return [
        {"x": np.ascontiguousarray(np.asarray(x, np.float32)[i * BPC:(i + 1) * BPC]),
         "w": wp, "b": bp}
        for i in range(N_CORES)
    ]


def kernel(x: np.ndarray, w: np.ndarray, b: np.ndarray) -> np.ndarray:
    nc = _get_nc(1)
    res = run_bass_kernel_spmd(nc, make_in_maps(x, w, b), list(range(N_CORES)))
    return np.concatenate([res.results[i]["y"] for i in range(N_CORES)],
                          axis=0).astype(np.float32)


def make_runner(nc, in_maps):
    """Hoisted version of bass2jax.run_bass_via_pjrt: build the sharded jit
    once with device-resident operands; returns (run_async, block) for
    throughput timing."""
    import jax
    from concourse import bass2jax
    from jax.sharding import Mesh, PartitionSpec, NamedSharding
    from jax.experimental.shard_map import shard_map

    bass2jax.install_neuronx_cc_hook()
    partition_name = nc.partition_id_tensor.name if nc.partition_id_tensor else None
    in_names, out_names, out_avals, zero_outs = [], [], [], []
    for alloc in nc.m.functions[0].allocations:
        if not isinstance(alloc, mybir.MemoryLocationSet):
            continue
        name = alloc.memorylocations[0].name
        if alloc.kind == "ExternalInput":
            if name != partition_name:
                in_names.append(name)
        elif alloc.kind == "ExternalOutput":
            shape = tuple(alloc.tensor_shape)
            dtype = mybir.dt.np(alloc.dtype)
            out_names.append(name)
            out_avals.append(jax.core.ShapedArray(shape, dtype))
            zero_outs.append(np.zeros(shape, dtype))
    n_params = len(in_names)
    all_in_names = list(in_names) + list(out_names)
    if partition_name is not None:
        all_in_names.append(partition_name)

    def _body(*args):
        operands = list(args)
        if partition_name is not None:
            operands.append(bass2jax.partition_id_tensor())
        return tuple(bass2jax._bass_exec_p.bind(
            *operands,
            out_avals=tuple(out_avals),
            in_names=tuple(all_in_names),
            out_names=tuple(out_names),
            lowering_input_output_aliases=(),
            sim_require_finite=True,
            sim_require_nnan=True,
            nc=nc,
        ))

    devices = jax.devices()[:N_CORES]
    mesh = Mesh(np.asarray(devices), ("core",))
    sharded = jax.jit(
        shard_map(_body, mesh=mesh,
                  in_specs=(PartitionSpec("core"),) * (n_params + len(out_names)),
                  out_specs=(PartitionSpec("core"),) * len(out_names),
                  check_rep=False),
        donate_argnums=(), keep_unused=True)
    sh = NamedSharding(mesh, PartitionSpec("core"))
    dev_in = [jax.device_put(np.concatenate(
        [np.asarray(in_maps[c][nm]) for c in range(N_CORES)], axis=0), sh)
        for nm in in_names]
    dev_zeros = [jax.device_put(
        np.zeros((N_CORES * z.shape[0], *z.shape[1:]), z.dtype), sh)
        for z in zero_outs]

    def run_async():
        return sharded(*dev_in, *dev_zeros)

    def block(out):
        return jax.block_until_ready(out)

    return run_async, block, out_names



# revision 2
# speedup vs baseline: 1.0021x; 1.0021x over previous
"""DownSample (depthwise FIR [1,3,3,1]^2/64 pad-2, then 3x3 stride-2 conv + bias)
as a Trainium2 Bass kernel, data-parallel over batch across 8 NeuronCores.

Block-pipelined redesign of the staged baseline:
  - per (batch, ic-chunk) block: ACT ingest cast -> DVE vertical cascade
    (c1, c2, V: three [1,1] stages = [1,3,3,1]) -> horizontal [1,2,1] as two
    plain [1,1] cascade adds (s, M) with the outer [1,1] folded into the conv
    weights (w4, 12 taps) -- no scaled-copy pass (the baseline burned an ACT
    pass on M = u + 2V; s/M cascade computes the identical M).
  - s is split between GPSIMD (rows < POOL_ROWS) and DVE (rest): GPSIMD is
    otherwise idle and runs tensor_add at ~2.2 ns/elem.
  - conv for block i-1 (48 N=512 matmuls) runs while block i's FIR computes;
    M is triple-buffered, PSUM groups accumulate across the two ic-chunk
    blocks of a batch (8 banks total).
  - pad rows/cols of the fixed ping-pong buffers are zeroed once in the
    preamble, not per block.
  - the V and M polyphase even/odd instruction pairs are each fused into
    ONE DVE instruction (4D / overlapping / stride-0-broadcast access
    patterns): HW-measured strided tensor_add runs at 0.59 ns/elem + 137
    ns/instr, so halving the instruction count on these stages saves
    ~0.5 us/block-pair (~2 us/rep, confirmed by same-process A/B).
  - the For_i timing loop wraps `reps` unrolled bodies so the per-iteration
    all-engine barrier amortizes.
"""

import numpy as np
import ml_dtypes

import concourse.bass as bass
import concourse.mybir as mybir
import concourse.tile as tile
from concourse.bass_utils import run_bass_kernel_spmd
from concourse.vector_clock import ScopedClock, VectorClock
from concourse.ap import AP

# problem geometry (hardcoded per contract)
B_FULL, C, H, W = 16, 256, 64, 64
OC, OH, OW = 256, 32, 32
N_CORES = 8
BPC = B_FULL // N_CORES      # batches per core
KH, KQ = 3, 4                # folded conv taps (rows x cols)
NCH = C // 128               # input-channel chunks
NOB = OC // 128              # output-channel blocks
WV = W + 4                   # padded V row width
WS = W + 3                   # s row width (67)
WM = W + 2                   # M row width (66)
NV = H + 1                   # V/s/M row count (65)
POOL_ROWS = 0                # s rows on GPSIMD (rest DVE)
# (GPSIMD offload measured slower on HW than the cost model predicts --
#  bf16 tensor_tensor on Q7 + SBUF-port contention with DVE; keep off.)

F32 = mybir.dt.float32
BF16 = mybir.dt.bfloat16
COPY = mybir.ActivationFunctionType.Copy
IDENT = mybir.ActivationFunctionType.Identity


class SplitDrainTileContext(tile.TileContext):
    """walrus codegen caps sync-wait commands per instruction; the kernel-tail
    drain waits once per proc lane, which overflows once several DMA queues
    are used. Emit one single-wait SP nop per pending proc first; SP executes
    in order, so the drain itself then needs no waits."""

    def _drain_and_barrier(self, tick_clock, wait_clock):
        g = tick_clock.global_clock
        n = len(g)
        for p in range(n):
            if g[p] > 0:
                vec = [g[q] if q == p else 0 for q in range(n)]
                nop = self.nc.sync.nop()
                wait_clock.add_sem_waits(nop.ins, ScopedClock({None: VectorClock(vec)}))
        self.nc.sync.drain()
        self.nc.all_engine_barrier()
        assert self.sems is not None
        popped = self.nc._tile_sem_poison_stack.pop()
        assert popped is self._sem_poison
        self.nc.clear_and_free_semaphores(list(self.sems.allocated().values()))
        self.nc.all_engine_barrier()


def _split_excess_waits(nc: bass.Bass, max_waits: int = 1) -> None:
    """The TRN2 ISA (and walrus codegen) allows at most 2 sync-wait commands
    per instruction (1 for matmul), but Tile's wait assignment can attach
    more. Spill excess waits onto same-engine nops inserted immediately
    before the instruction."""
    for fn in nc.m.functions:
        for bb in fn.blocks:
            out = []
            changed = False
            for ins in bb.instructions:
                si = ins.sync_info
                waits = list(si.on_wait) if si else []
                if len(waits) > max_waits:
                    changed = True
                    excess, keep = waits[:-max_waits], waits[-max_waits:]
                    for k in range(0, len(excess), max_waits):
                        nop = mybir.InstNoOp(
                            name=f"{ins.name}-wsplit{k}", ins=[], outs=[],
                            engine=ins.engine)
                        nop.sync_info = mybir.SyncInfo(
                            on_wait=excess[k:k + max_waits], on_update=[])
                        nc.register_instruction(nop, overwrite=True)
                        out.append(nop)
                    ins.sync_info = mybir.SyncInfo(
                        on_wait=keep, on_update=list(si.on_update))
                out.append(ins)
            if changed:
                bb.instructions = out


def build_program(reps: int = 1, loop_n: int | None = None,
                  staggered: bool = False) -> bass.Bass:
    """One-core program; run SPMD on 8 cores. `reps` unrolls the body; with
    `loop_n` a single device-side For_i wraps all `reps` bodies."""
    nc = bass.Bass()
    xin = nc.declare_dram_parameter("x", [BPC, C, H, W], F32, isOutput=False)
    win = nc.declare_dram_parameter("w", [128, KH * KQ * NCH * NOB * 128], BF16,
                                    isOutput=False)
    bin_ = nc.declare_dram_parameter("b", [NOB, 128], F32, isOutput=False)
    yout = nc.declare_dram_parameter("y", [BPC, OC, OH, OW], F32, isOutput=True)

    blocks = [(b, c) for b in range(BPC) for c in range(NCH)]
    NB = len(blocks)

    with SplitDrainTileContext(nc) as tc:
        rotated = loop_n is not None
        with tc.tile_pool(name="const", bufs=1) as cpool, \
             tc.tile_pool(name="xf", bufs=3) as xfpool, \
             tc.tile_pool(name="fir", bufs=2) as fpool, \
             tc.tile_pool(name="vsm", bufs=3) as vpool, \
             tc.tile_pool(name="mp", bufs=4) as mpool, \
             tc.tile_pool(name="osb", bufs=3) as opool, \
             tc.tile_pool(name="ps", bufs=4 if rotated else 8,
                          space="PSUM") as pspool, \
             tc.tile_pool(name="psl", bufs=1, space="PSUM") as pslpool:

            w_sb = cpool.tile([128, KH * KQ * NCH * NOB * 128], BF16)
            nc.sync.dma_start(out=w_sb[:], in_=win[:])
            bias_sb = cpool.tile([128, NOB], F32)
            nc.sync.dma_start(out=bias_sb[:], in_=bin_[:].rearrange("o p -> p o"))

            # Column-polyphase layout: every image buffer stores even columns
            # in its left half and odd columns in its right half, so all DVE
            # adds stay packed (2x mode) AND the conv matmul rhs reads
            # CONTIGUOUS column windows (strided last-dim rhs measured ~25
            # ns/MM slower on HW).
            #   xb/c1/c2 [rows, 64]: cols 0:32 = x even cols, 32:64 = odd
            #   V [65, 68]: Ve = cols 0:34 (Vb even), Vo = 34:68 (Vb odd)
            #   s [65, 67]: Se = cols 0:34 (Sb even), So = 34:67 (Sb odd)
            #   m [65, 66]: Me = cols 0:33 (Mb even), Mo = 33:66 (Mb odd)
            # with Sb[g] = Vb[g]+Vb[g+1], Mb[g] = Sb[g]+Sb[g+1] as before:
            #   Se[j] = Ve[j]+Vo[j]      So[j] = Vo[j]+Ve[j+1]
            #   Me[j] = Se[j]+So[j]      Mo[j] = So[j]+Se[j+1]
            # conv tap q reads Mb[q+2ow]: q=0 -> Me[ow], q=1 -> Mo[ow],
            # q=2 -> Me[ow+1], q=3 -> Mo[ow+1].

            # fixed 3-deep rotations for the two pad-carrying buffers; pads
            # are zeroed once here instead of per block.
            HW2 = W // 2  # 32
            xb3f, v3f, v4f = [], [], []
            for j in range(3):
                xb = cpool.tile([128, (H + 4) * W], BF16, name=f"xb{j}")
                xb3 = xb[:].rearrange("p (h w) -> p h w", w=W)
                # full-tile zero: covers the pad rows AND gives iteration 0
                # of the rotated loop defined block-0 data
                nc.vector.memset(xb3[:, :, :], 0.0)
                xb3f.append(xb3)
                vb = cpool.tile([128, NV * WV], BF16, name=f"v{j}")
                v3 = vb[:].rearrange("p (h w) -> p h w", w=WV)
                # Vb zero pads: Ve[0]=Vb[0], Vo[0]=Vb[1], Ve[33]=Vb[66],
                # Vo[33]=Vb[67] -> polyphase cols 0, 33, 34, 67
                nc.vector.memset(v3[:, :, 0:1], 0.0)
                nc.vector.memset(v3[:, :, 33:35], 0.0)
                nc.vector.memset(v3[:, :, 67:68], 0.0)
                v3f.append(v3)
                v4f.append(vb[:].rearrange(
                    "p (h a w) -> p h a w", h=NV, a=2, w=34))

            xb3s: dict = {}
            v3s: dict = {}
            s3s: dict = {}
            s3s_base: dict = {}
            m3s: dict = {}

            def emit_load(rep, i):
                b, ci = blocks[i]
                xb3 = xb3f[(rep * NB + i) % 3]
                for (r0, nr) in ((0, 32), (32, 32)):
                    xf = xfpool.tile([128, nr * W], F32, tag="xf",
                                     name=f"xf{rep}_{i}_{r0}")
                    nc.sync.dma_start(
                        out=xf[:],
                        in_=xin[b, ci * 128:(ci + 1) * 128, r0:r0 + nr, :]
                        .rearrange("c h w -> c (h w)"))
                    xf3 = xf[:].rearrange("p (h w) -> p h w", w=W)
                    # cast + polyphase split (ACT is stride-insensitive)
                    nc.scalar.activation(
                        xb3[:, 2 + r0:2 + r0 + nr, 0:HW2],
                        xf3[:, :, 0:W:2], COPY)
                    nc.scalar.activation(
                        xb3[:, 2 + r0:2 + r0 + nr, HW2:W],
                        xf3[:, :, 1:W:2], COPY)
                xb3s[i % 2] = xb3

            def emit_fir_v(rep, i):
                xb3 = xb3s[i % 2]
                c1 = fpool.tile([128, (H + 3) * W], BF16, tag="c1",
                                name=f"c1_{rep}_{i}")
                c13 = c1[:].rearrange("p (h w) -> p h w", w=W)
                c2 = fpool.tile([128, (H + 2) * W], BF16, tag="c2",
                                name=f"c2_{rep}_{i}")
                c23 = c2[:].rearrange("p (h w) -> p h w", w=W)
                v3 = v3f[(rep * NB + i) % 3]
                # vertical [1,3,3,1] as three [1,1] cascade stages, on the
                # even and odd column halves (same total elements)
                nc.vector.tensor_add(c13[:, 0:H + 3, :],
                                     xb3[:, 0:H + 3, :],
                                     xb3[:, 1:H + 4, :])
                nc.vector.tensor_add(c23[:, 0:H + 2, :],
                                     c13[:, 0:H + 2, :],
                                     c13[:, 1:H + 3, :])
                # V data: Ve cols 1..32 <- c2 even half, Vo cols 1..32
                # <- odd half, as ONE 4D instruction (even and odd halves of
                # v/c2 are congruent windows; pads were pre-zeroed above)
                v4 = v4f[(rep * NB + i) % 3]
                c24 = c2[:].rearrange("p (h a w) -> p h a w",
                                      h=H + 2, a=2, w=HW2)
                nc.vector.tensor_add(v4[:, 0:NV, :, 1:33],
                                     c24[:, 0:NV, :, :],
                                     c24[:, 1:NV + 1, :, :])
                v3s[i % 2] = v3

            def emit_fir_s(rep, i):
                v3 = v3s[i % 2]
                sb = vpool.tile([128, NV * WS], BF16, tag="s",
                                name=f"s_{rep}_{i}")
                s3 = sb[:].rearrange("p (h w) -> p h w", w=WS)
                # Se[j] = Ve[j]+Vo[j] (j 0..33); So[j] = Vo[j]+Ve[j+1] (0..32)
                if POOL_ROWS > 0:
                    nc.gpsimd.tensor_add(s3[:, 0:POOL_ROWS, 0:34],
                                         v3[:, 0:POOL_ROWS, 0:34],
                                         v3[:, 0:POOL_ROWS, 34:68])
                if POOL_ROWS < NV:
                    nc.vector.tensor_add(s3[:, POOL_ROWS:NV, 0:34],
                                         v3[:, POOL_ROWS:NV, 0:34],
                                         v3[:, POOL_ROWS:NV, 34:68])
                nc.vector.tensor_add(s3[:, 0:NV, 34:67],
                                     v3[:, 0:NV, 34:67],
                                     v3[:, 0:NV, 1:34])
                s3s[i % 2] = s3
                s3s_base[i % 2] = sb[:]

            def emit_fir_m(rep, i, mb=None):
                s3 = s3s[i % 2]
                if mb is None:
                    mb = mpool.tile([128, NV * WM], BF16, tag="m",
                                    name=f"m_{rep}_{i}")
                m3 = mb[:].rearrange("p (h w) -> p h w", w=WM)
                # Me[j] = Se[j]+So[j] (j 0..32); Mo[j] = So[j]+Se[j+1]
                # (0..32), as ONE instruction: out = [Me(33) | Mo(33)]
                # = [[33,2],[1,33]]; in0 = [Se[0:33] | Se[1:34]] overlapping
                # [[1,2],[1,33]]; in1 = So broadcast [[0,2],[1,33]].
                mB = mb[:]
                sB = s3s_base[i % 2]
                pm = list(mB.ap[0])
                ps_ = list(sB.ap[0])
                out4 = AP(mB.tensor, mB.offset,
                          [pm, [WM, NV], [33, 2], [1, 33]])
                in0 = AP(sB.tensor, sB.offset,
                         [ps_, [WS, NV], [1, 2], [1, 33]])
                in1 = AP(sB.tensor, sB.offset + 34,
                         [ps_, [WS, NV], [0, 2], [1, 33]])
                nc.vector.tensor_add(out4, in0, in1)
                m3s[i % 3] = m3

            def emit_conv(pss, i, skip_check=False):
                b, ci = blocks[i]
                mm = m3s[i % 3]
                for grp in range(2):
                    for ocb in range(NOB):
                        for kh in range(KH):
                            # even-parity taps first: they read only Me,
                            # which the FIR writes before Mo
                            for q in (0, 2, 1, 3):
                                widx = ((kh * KQ + q) * NCH + ci) * NOB + ocb
                                lhsT = w_sb[:, widx * 128:(widx + 1) * 128]
                                first = ci == 0 and kh == 0 and q == 0
                                last = (ci == NCH - 1 and kh == KH - 1
                                        and q == KQ - 1)
                                c0 = 33 * (q % 2) + q // 2
                                rhs = mm[:, 32 * grp + kh:32 * grp + kh + 31:2,
                                         c0:c0 + 32]
                                out3 = pss[(b, ocb, grp)][:].rearrange(
                                    "p (h w) -> p h w", w=OW)
                                nc.tensor.matmul(out3, lhsT, rhs,
                                                 start=first, stop=last,
                                                 skip_group_check=skip_check)

            def emit_evac(rep, pss, b):
                for ocb in range(NOB):
                    osb = opool.tile([128, OH * OW], F32, tag="osb",
                                     name=f"osb{rep}_{b}_{ocb}")
                    for grp in range(2):
                        nc.scalar.activation(
                            osb[:, grp * 512:(grp + 1) * 512],
                            pss[(b, ocb, grp)][:],
                            IDENT, bias=bias_sb[:, ocb:ocb + 1])
                    nc.sync.dma_start(
                        out=yout[b, ocb * 128:(ocb + 1) * 128, :, :]
                        .rearrange("c h w -> c (h w)"),
                        in_=osb[:])

            def emit_warmup(n_mm, pss):
                # PE warmup garbage matmuls into a bank the real taps later
                # clear (start=True); keeps HAM warming during the FIR lead-in.
                wout = pss[(0, 0, 0)][:].rearrange("p (h w) -> p h w", w=OW)
                rhs = xb3s[0][:, 0:16, 0:32]
                for _ in range(n_mm):
                    nc.tensor.matmul(wout, w_sb[:, 0:128], rhs,
                                     start=True, stop=True)

            # In the loop build (`rotated`), the last rep's final conv +
            # b1 evacuation move to the TOP of the body: they consume the
            # loop-carried M tile / PSUM partials written at the body's end,
            # so right after the For_i all-engine barrier every engine has
            # immediate work (no serial fill behind the previous tail).
            m_last = None
            psl = None
            if rotated:
                m_last = cpool.tile([128, NV * WM], BF16, name="mlast")
                nc.vector.memset(m_last[:], 0.0)
                psl = {(1, ocb, grp): pslpool.tile(
                    [128, 512], F32, name=f"psl_{ocb}_{grp}")
                    for ocb in range(NOB) for grp in range(2)}

            _loop = None
            if loop_n is not None:
                _loop = tc.For_i(0, loop_n, 1, hint_engines=(
                    mybir.EngineType.PE, mybir.EngineType.DVE,
                    mybir.EngineType.Activation, mybir.EngineType.Pool),
                    staggered_reset=staggered)
                _loop.__enter__()

            if rotated:
                # finish the previous iteration's b1: stop-half of its conv
                # accumulation group (PSUM state persists across the
                # barrier); the evac is deferred past rep 0's first casts.
                # Iteration 0 computes garbage into y[b1], overwritten by
                # every later iteration.
                m3s[(NB - 1) % 3] = m_last[:].rearrange(
                    "p (h w) -> p h w", w=WM)
                emit_conv(psl, NB - 1, skip_check=True)

            # b1's evacuation of inner reps is deferred into the NEXT rep's
            # stream (after its first ingest cast) so the next rep's ACT/DVE
            # work is not queued behind an evac waiting on this rep's conv.
            # In rotated builds the block-0 loads are ALSO shifted one rep
            # back (the last rep prefetches the next ITERATION's block 0
            # across the barrier into the xb ring -- requires reps*NB % 3
            # == 0 so the ring phase matches).
            if rotated:
                assert (reps * NB) % 3 == 0, "rotated loads need reps%3==0"
            pending_evac = ("rot", psl) if rotated else None
            for rep in range(reps):
                last = rep == reps - 1
                pss = {(b, ocb, grp): (
                    psl[(b, ocb, grp)] if (rotated and b == 1)
                    else pspool.tile([128, 512], F32, tag="ps",
                                     name=f"ps{rep}_{b}_{ocb}_{grp}"))
                    for b in range(BPC) for ocb in range(NOB)
                    for grp in range(2)}
                if not rotated:
                    emit_load(rep, 0)
                elif rep == 0:
                    # block-0 data was prefetched by the previous iteration's
                    # tail; just point xb3s at the right ring slot.
                    xb3s[0] = xb3f[0]
                if rep == 0 and loop_n is None:
                    emit_warmup(12, pss)
                for i in range(NB):
                    if i >= 1:
                        # M(i-1) first: it only needs s(i-1), and emitting it
                        # ahead of block i's cascade lets PE start conv(i-1)
                        # a couple of microseconds earlier.
                        emit_fir_m(rep, i - 1)
                        emit_conv(pss, i - 1, skip_check=rotated and i - 1 >= 2)
                    if i + 1 < NB:
                        emit_load(rep, i + 1)
                    elif rotated:
                        if not last:
                            emit_load(rep + 1, 0)
                        else:
                            emit_load(0, 0)  # next iteration's block 0
                    if i == 0 and pending_evac is not None:
                        # deferred b1 evac AFTER this segment's casts: the
                        # in-order ACT queue must not hold the next block's
                        # ingest cast behind an evac that waits on a conv.
                        emit_evac(*pending_evac, 1)
                        pending_evac = None
                    if i == 2:
                        emit_evac(rep, pss, 0)
                    emit_fir_v(rep, i)
                    emit_fir_s(rep, i)
                if rotated and last:
                    # M(3) lands in the loop-carried tile; its conv + evac
                    # run at the next iteration's top.
                    emit_fir_m(rep, NB - 1, mb=m_last)
                else:
                    emit_fir_m(rep, NB - 1)
                    emit_conv(pss, NB - 1, skip_check=rotated)
                    pending_evac = (rep, pss)
            if pending_evac is not None:
                emit_evac(*pending_evac, 1)

            if _loop is not None:
                _loop.__exit__(None, None, None)
    _split_excess_waits(nc)
    return nc


def prep_weights(w: np.ndarray) -> np.ndarray:
    """w [256,256,3,3] f32 -> [128, 48*128] bf16 lhsT tiles.
    Folds horizontal [1,1] and the 1/64 FIR normalization:
    w4[q] coefficients multiply M[2ow+q-1]."""
    w = np.asarray(w, np.float32)
    w4 = np.zeros((OC, C, KH, KQ), np.float32)
    w4[:, :, :, 0] = w[:, :, :, 0]
    w4[:, :, :, 1] = w[:, :, :, 0] + w[:, :, :, 1]
    w4[:, :, :, 2] = w[:, :, :, 1] + w[:, :, :, 2]
    w4[:, :, :, 3] = w[:, :, :, 2]
    w4 *= 1.0 / 64.0
    # -> [kh, q, c_chunk, ocb, ic(128), oc(128)]
    t = w4.reshape(NOB, 128, NCH, 128, KH, KQ).transpose(4, 5, 2, 0, 3, 1)
    t = np.ascontiguousarray(t).reshape(KH * KQ * NCH * NOB, 128, 128)
    return t.transpose(1, 0, 2).reshape(128, -1).astype(ml_dtypes.bfloat16)


_NC_CACHE: dict = {}


def _get_nc(reps: int = 1, loop_n: int | None = None,
            staggered: bool = False) -> bass.Bass:
    key = (reps, loop_n, staggered)
    if key not in _NC_CACHE:
        _NC_CACHE[key] = build_program(reps, loop_n, staggered)
    return _NC_CACHE[key]


def make_in_maps(x: np.ndarray, w: np.ndarray, b: np.ndarray):
    wp = prep_weights(w)
    bp = np.asarray(b, np.float32).reshape(NOB, 128)
    return [
        {"x": np.ascontiguousarray(np.asarray(x, np.float32)[i * BPC:(i + 1) * BPC]),
         "w": wp, "b": bp}
        for i in range(N_CORES)
    ]


def kernel(x: np.ndarray, w: np.ndarray, b: np.ndarray) -> np.ndarray:
    nc = _get_nc(1)
    res = run_bass_kernel_spmd(nc, make_in_maps(x, w, b), list(range(N_CORES)))
    return np.concatenate([res.results[i]["y"] for i in range(N_CORES)],
                          axis=0).astype(np.float32)


def make_runner(nc, in_maps):
    """Hoisted version of bass2jax.run_bass_via_pjrt: build the sharded jit
    once with device-resident operands; returns (run_async, block) for
    throughput timing."""
    import jax
    from concourse import bass2jax
    from jax.sharding import Mesh, PartitionSpec, NamedSharding
    from jax.experimental.shard_map import shard_map

    bass2jax.install_neuronx_cc_hook()
    partition_name = nc.partition_id_tensor.name if nc.partition_id_tensor else None
    in_names, out_names, out_avals, zero_outs = [], [], [], []
    for alloc in nc.m.functions[0].allocations:
        if not isinstance(alloc, mybir.MemoryLocationSet):
            continue
        name = alloc.memorylocations[0].name
        if alloc.kind == "ExternalInput":
            if name != partition_name:
                in_names.append(name)
        elif alloc.kind == "ExternalOutput":
            shape = tuple(alloc.tensor_shape)
            dtype = mybir.dt.np(alloc.dtype)
            out_names.append(name)
            out_avals.append(jax.core.ShapedArray(shape, dtype))
            zero_outs.append(np.zeros(shape, dtype))
    n_params = len(in_names)
    all_in_names = list(in_names) + list(out_names)
    if partition_name is not None:
        all_in_names.append(partition_name)

    def _body(*args):
        operands = list(args)
        if partition_name is not None:
            operands.append(bass2jax.partition_id_tensor())
        return tuple(bass2jax._bass_exec_p.bind(
            *operands,
            out_avals=tuple(out_avals),
            in_names=tuple(all_in_names),
            out_names=tuple(out_names),
            lowering_input_output_aliases=(),
            sim_require_finite=True,
            sim_require_nnan=True,
            nc=nc,
        ))

    devices = jax.devices()[:N_CORES]
    mesh = Mesh(np.asarray(devices), ("core",))
    sharded = jax.jit(
        shard_map(_body, mesh=mesh,
                  in_specs=(PartitionSpec("core"),) * (n_params + len(out_names)),
                  out_specs=(PartitionSpec("core"),) * len(out_names),
                  check_rep=False),
        donate_argnums=(), keep_unused=True)
    sh = NamedSharding(mesh, PartitionSpec("core"))
    dev_in = [jax.device_put(np.concatenate(
        [np.asarray(in_maps[c][nm]) for c in range(N_CORES)], axis=0), sh)
        for nm in in_names]
    dev_zeros = [jax.device_put(
        np.zeros((N_CORES * z.shape[0], *z.shape[1:]), z.dtype), sh)
        for z in zero_outs]

    def run_async():
        return sharded(*dev_in, *dev_zeros)

    def block(out):
        return jax.block_until_ready(out)

    return run_async, block, out_names

